# revision 8
# baseline (speedup 1.0000x reference)
"""DiT block kernel for 8 TRN2 NeuronCores (self-contained).

Sharding: cores 0-3 <-> batch 0, cores 4-7 <-> batch 1.
Per 4-core group: attention head-parallel (3 of 12 heads/core, all 2048
tokens), W_o row-sharded -> split ReduceScatter (2 halves, bf16) -> each
core owns 2x256-token slices; FFN token-parallel (512 rows, full weights
prefetched to SBUF in bf16). AdaLN/cond path is DH-sharded over all 8
cores with host-folded (cond_w2 @ W_mod) matrices in row layout -> one
small AllReduce overlapped with the rms-stats phase. All big matmuls run
bf16 (1 PE cycle/row, half DMA), accumulation in fp32 PSUM.
"""
import numpy as np
import ml_dtypes

import concourse.bass as bass
import concourse.mybir as mybir
import concourse.tile as tile
from concourse import bacc, bass_utils
from concourse.masks import make_identity

FP32 = mybir.dt.float32
FP32R = mybir.dt.float32r
BF16 = mybir.dt.bfloat16
AF = mybir.ActivationFunctionType
ALU = mybir.AluOpType
AX = mybir.AxisListType

NPBF = ml_dtypes.bfloat16

B, L, D, H, DH = 2, 2048, 768, 12, 3072
HD = 64
EPS = 1e-6
SCALE = float(np.sqrt(HD))
NC_ = 8
G = 4            # cores per batch group
HC = 3           # heads per core
TOK = L // G     # 512
HTOK = TOK // 2  # 256: tokens per RS half
DH8 = DH // NC_  # 384
GROUPS = [[0, 1, 2, 3], [4, 5, 6, 7]]
KC = L // 128    # 16 key chunks
JT = L // 512    # 4 q tiles
DK = D // 128    # 6 d chunks
MG = DH // 128   # 24 dh chunks


def _bf(a):
    return np.ascontiguousarray(np.asarray(a, np.float32)).astype(NPBF)


# ---------------------------------------------------------------- host prep
def host_prep(inp):
    f = {k: np.ascontiguousarray(np.asarray(v, np.float32)) for k, v in inp.items()}
    x, c = f["x"], f["c"]
    cos, sin = f["freqs_cos"], f["freqs_sin"]          # [L, 32]

    attn_gamma_s = f["attn_gamma"] * f["attn_norm_w"][None, :]
    ffn_gamma_s = f["ffn_gamma"] * f["ffn_norm_w"][None, :]
    mods = [attn_gamma_s, f["attn_beta"], f["attn_alpha"],
            ffn_gamma_s, f["ffn_beta"], f["ffn_gamma"]]
    wfold_full = [f["cond_w2"] @ m for m in mods]       # [DH, D] x6
    bvec = np.stack([f["cond_b2"] @ m for m in mods])   # [6, D]

    perm = np.concatenate([np.arange(0, HD, 2), np.arange(1, HD, 2)])
    cosT, sinT = cos.T, sin.T                            # [32, L]
    cct = np.tile(cosT, (4, 1)).astype(np.float32)       # [128, L]
    sst = np.concatenate([-sinT, sinT, -sinT, sinT], 0).astype(np.float32)

    cT = np.ascontiguousarray(c.T)                       # [768, 2]
    ct_pack = cT.reshape(6, 128, 2).transpose(1, 0, 2).reshape(128, 12).copy()

    wg_blk = f["ffn_gate"].reshape(6, 128, 24, 128).transpose(2, 1, 0, 3) \
        .reshape(24 * 128, 768).copy()
    wh_blk = f["ffn_hidden"].reshape(6, 128, 24, 128).transpose(2, 1, 0, 3) \
        .reshape(24 * 128, 768).copy()

    pswap = np.zeros((128, 128), np.float32)
    for i_ in range(128):
        pswap[i_, i_ ^ 32] = 1.0

    cores = []
    for i in range(NC_):
        g, r = i // G, i % G
        hs = [HC * r + j for j in range(HC)]
        si = slice(DH8 * i, DH8 * (i + 1))
        blocks = [f["W_q"][:, h * HD:(h + 1) * HD][:, perm] for h in hs]
        blocks += [f["W_k"][:, h * HD:(h + 1) * HD][:, perm] for h in hs]
        wqk = np.concatenate(blocks, 1)                  # [768, 384]
        wv = np.concatenate(
            [f["W_v"][:, h * HD:(h + 1) * HD] for h in hs], 1)  # [768, 192]
        wo = np.concatenate([f["W_o"][h * HD:(h + 1) * HD] for h in hs], 0)

        # token ownership: {256r..256r+255} U {1024+256r..1024+256r+255}
        tsl0 = slice(HTOK * r, HTOK * (r + 1))
        tsl1 = slice(L // 2 + HTOK * r, L // 2 + HTOK * (r + 1))
        x_slice = np.concatenate([x[g, tsl0], x[g, tsl1]], 0)   # [512, 768]

        gsel = np.zeros((2, 1), np.float32)
        gsel[g, 0] = 1.0

        cores.append(dict(
            xT=_bf(x[g].T),
            x_slice=np.ascontiguousarray(x_slice),
            cct=_bf(cct), sst=_bf(sst),
            ct_pack=_bf(ct_pack),
            w1s=_bf(f["cond_w1"][:, si]),
            b1_rows=np.ascontiguousarray(
                np.tile(f["cond_b1"][si][None, :], (2, 1))),    # [2, 384]
            wfold=_bf(np.concatenate([w[si] for w in wfold_full], 0)),
            gsel=_bf(gsel),
            bv_cat=np.ascontiguousarray(bvec.reshape(1, 6 * D)),
            pswap=_bf(pswap),
            wqk=_bf(wqk), wv=_bf(wv), wo=_bf(wo),
            wg_blk=_bf(wg_blk), wh_blk=_bf(wh_blk), wout=_bf(f["ffn_out"]),
        ))
    return cores


# ---------------------------------------------------------------- program
_CACHE = {}

DRAM_SPECS = [
    ("xT", [D, L], BF16),
    ("x_slice", [TOK, D], FP32),
    ("cct", [128, L], BF16),
    ("sst", [128, L], BF16),
    ("ct_pack", [128, 12], BF16),
    ("w1s", [D, DH8], BF16),
    ("b1_rows", [2, DH8], FP32),
    ("wfold", [6 * DH8, D], BF16),
    ("gsel", [2, 1], BF16),
    ("bv_cat", [1, 6 * D], FP32),
    ("pswap", [128, 128], BF16),
    ("wqk", [D, 384], BF16),
    ("wv", [D, HC * HD], BF16),
    ("wo", [HC * HD, D], BF16),
    ("wg_blk", [MG * 128, D], BF16),
    ("wh_blk", [MG * 128, D], BF16),
    ("wout", [DH, D], BF16),
]


def build_program(reps=1):
    nc = bacc.Bacc("TRN2", target_bir_lowering=False, debug=False,
                   num_devices=NC_)
    dr = {}
    for name, shape, dt in DRAM_SPECS:
        dr[name] = nc.dram_tensor(name, shape, dt, kind="ExternalInput")
    out_d = nc.dram_tensor("out", [TOK, D], FP32, kind="ExternalOutput")

    with tile.TileContext(nc) as tc, \
         nc.allow_low_precision(reason="bf16 matmuls, fp32 PSUM accumulation"):
        for _ in range(reps):
            _emit(nc, tc, dr, out_d)
    nc.compile()
    return nc


def _phase_a1(nc, tc, dr, st):
    """cond MLP partial (row layout) -> ar_in; issue AllReduce."""
    with tc.tile_pool(name="pa", bufs=1) as pa, \
         tc.tile_pool(name="pa_wf", bufs=6) as pa_wf, \
         tc.tile_pool(name="pa_ps", bufs=2, space="PSUM") as pa_ps:
        ct_sb = pa.tile([128, 12], BF16, name="ct_sb")
        nc.sync.dma_start(ct_sb[:], dr["ct_pack"].ap())
        b1_sb = pa.tile([2, DH8], FP32, name="b1_sb")
        nc.sync.dma_start(b1_sb[:], dr["b1_rows"].ap())
        w1_sb = [pa.tile([128, DH8], BF16, name=f"w1_sb{k}")
                 for k in range(DK)]
        for k in range(DK):
            nc.sync.dma_start(w1_sb[k][:],
                              dr["w1s"].ap()[128 * k:128 * (k + 1), :])

        # h1 [2, 384] = c @ W1s  (both batches)
        h1_ps = pa_ps.tile([2, DH8], FP32, name="h1_ps")
        for k in range(DK):
            nc.tensor.matmul(h1_ps[:], ct_sb[:, 2 * k:2 * k + 2],
                             w1_sb[k][:],
                             start=(k == 0), stop=(k == DK - 1))
        h1_sb = pa.tile([2, DH8], FP32, name="h1_sb")
        nc.vector.tensor_add(h1_sb[:], h1_ps[:], b1_sb[:])
        silu_r = pa.tile([2, DH8], BF16, name="silu_r")
        nc.scalar.activation(silu_r[:], h1_sb[:], AF.Silu)
        # silu columns [128, 2] x3 chunks
        silu_c = pa.tile([128, 6], BF16, name="silu_c")
        scp = pa_ps.tile([128, 6], BF16, name="scp")
        for cch in range(3):
            nc.tensor.transpose(scp[:, 2 * cch:2 * cch + 2],
                                silu_r[:, 128 * cch:128 * (cch + 1)],
                                st["ident_bf"][0:2, 0:2])
        nc.vector.tensor_copy(silu_c[:], scp[:])

        arin_sb = pa.tile([2, 6 * D], FP32, name="arin_sb")
        for m in range(6):
            for half in range(2):
                mp = pa_ps.tile([2, 384], FP32, name="mp")
                for k in range(3):
                    wf = pa_wf.tile([128, D], BF16, name="wf")
                    if half == 0:
                        nc.sync.dma_start(
                            wf[:], dr["wfold"].ap()[128 * (3 * m + k):
                                                    128 * (3 * m + k + 1), :])
                        st[f"wf{m}_{k}"] = wf
                    else:
                        wf = st[f"wf{m}_{k}"]
                    nc.tensor.matmul(mp[:],
                                     silu_c[:, 2 * k:2 * k + 2],
                                     wf[:, 384 * half:384 * (half + 1)],
                                     start=(k == 0), stop=(k == 2))
                nc.vector.tensor_copy(
                    arin_sb[0:2, m * D + 384 * half:
                            m * D + 384 * (half + 1)],
                    mp[:])
        nc.sync.dma_start(st["ar_in"][:], arin_sb[:])
        nc.gpsimd.collective_compute(
            "AllReduce", ALU.add, replica_groups=[list(range(NC_))],
            ins=[st["ar_in"].opt()], outs=[st["ar_out"].opt()])


def _phase_a2(nc, tc, dr, pers, st):
    """Consume AllReduce -> mod_cols (m=0,1) + bcast tiles (m=2..5)."""
    mod_cols, bcast = st["mod_cols"], st["bcast"]
    with tc.tile_pool(name="pa2", bufs=1) as pa2, \
         tc.tile_pool(name="pa2_ps", bufs=2, space="PSUM") as pa2_ps:
        gsel_sb = pa2.tile([2, 1], BF16, name="gsel_sb")
        nc.sync.dma_start(gsel_sb[:], dr["gsel"].ap())
        bv_sb = pa2.tile([1, 6 * D], FP32, name="bv_sb")
        nc.sync.dma_start(bv_sb[:], dr["bv_cat"].ap())
        ar_sb = pa2.tile([2, 6 * D], FP32, name="ar_sb")
        nc.sync.dma_start(ar_sb[:], st["ar_out"][:])
        ar_bf = pa2.tile([2, 6 * D], BF16, name="ar_bf")
        nc.vector.tensor_copy(ar_bf[:], ar_sb[:])

        # per-mod rows at partition 0 (batch row picked by gsel matmul)
        mod_bf = [pa2.tile([1, D], BF16, name=f"mod_bf{m}")
                  for m in range(6)]
        for m in range(6):
            for half in range(2):
                sl = slice(384 * half, 384 * (half + 1))
                sp = pa2_ps.tile([1, 384], FP32, name="sp")
                nc.tensor.matmul(sp[:], gsel_sb[:],
                                 ar_bf[0:2, m * D + 384 * half:
                                       m * D + 384 * (half + 1)])
                nc.vector.tensor_add(mod_bf[m][0:1, sl], sp[:],
                                     bv_sb[0:1, m * D + 384 * half:
                                           m * D + 384 * (half + 1)])

        # bcast tiles for m = 2..5
        for m in (2, 3, 4, 5):
            for half in range(2):
                sl = slice(384 * half, 384 * (half + 1))
                bp = pa2_ps.tile([128, 384], FP32, name="bp")
                nc.tensor.matmul(bp[:], st["ones_bf"][:],
                                 mod_bf[m][0:1, sl])
                nc.vector.tensor_copy(bcast[m][:, sl], bp[:])
        # mod columns for m = 0 (gamma) / m = 1 (beta); even psum cols so
        # each bf16 write stays 4-byte aligned
        mcp = pa2_ps.tile([128, 24], BF16, name="mcp")
        for m in range(2):
            for k in range(DK):
                col = 2 * (6 * m + k)
                nc.tensor.transpose(mcp[:, col:col + 1],
                                    mod_bf[m][0:1, 128 * k:128 * (k + 1)],
                                    st["ident_bf"][0:1, 0:1])
        nc.vector.tensor_copy(mod_cols[:], mcp[:, 0:24:2])


def _phase_b_stats(nc, tc, dr, st, xt, rb):
    """xT load + rms stats -> rb (bf16 reciprocal-rms broadcast)."""
    with tc.tile_pool(name="pb", bufs=2) as pb, \
         tc.tile_pool(name="pb_ps", bufs=1, space="PSUM") as pb_ps:
        for k in range(DK):
            nc.sync.dma_start(xt[k][:],
                              dr["xT"].ap()[128 * k:128 * (k + 1), :])
        msq = [pb_ps.tile([1, 512], FP32, name=f"msq{j}") for j in range(4)]
        for k in range(DK):
            for j in range(4):
                xsq = pb.tile([128, 512], BF16, name="xsq")
                nc.vector.tensor_mul(xsq[:], xt[k][:, 512 * j:512 * (j + 1)],
                                     xt[k][:, 512 * j:512 * (j + 1)])
                nc.tensor.matmul(msq[j][:], st["onescol_bf"][:], xsq[:],
                                 start=(k == 0), stop=(k == DK - 1))
        for j in range(4):
            sq_sb = pb.tile([1, 512], FP32, name="sq_sb")
            nc.scalar.activation(sq_sb[:], msq[j][:], AF.Sqrt,
                                 bias=st["eps_sb"][0:1, 0:1], scale=1.0 / D)
            rinv = pb.tile([1, 512], FP32R, name="rinv")
            nc.vector.reciprocal(rinv[:], sq_sb[:])
            rbp = pb_ps.tile([128, 512], FP32, name="rbp", bufs=2)
            nc.tensor.matmul(rbp[:], st["ones_r"][:], rinv[:])
            nc.vector.tensor_copy(rb[:, 512 * j:512 * (j + 1)], rbp[:])


def _phase_b_ht(nc, tc, st, xt, rb, hT):
    """hT = rms-normed, modulated x (bf16, transposed layout)."""
    mod_cols = st["mod_cols"]
    with tc.tile_pool(name="pbh", bufs=2) as pbh:
        for k in range(DK):
            for j in range(4):
                sl = slice(512 * j, 512 * (j + 1))
                tmp = pbh.tile([128, 512], FP32, name="tmp")
                nc.vector.tensor_mul(tmp[:], xt[k][:, sl], rb[:, sl])
                nc.vector.tensor_scalar(
                    hT[k][:, sl], tmp[:], mod_cols[:, k:k + 1],
                    mod_cols[:, 6 + k:7 + k], op0=ALU.mult, op1=ALU.add)


def _phase_c(nc, tc, dr, st, hT, qkr, v_sb):
    """QKV matmuls + RoPE + v_aug tiles (all bf16)."""
    with tc.tile_pool(name="pc_w", bufs=1) as pc_w, \
         tc.tile_pool(name="pc", bufs=2) as pc, \
         tc.tile_pool(name="pc_ps", bufs=2, space="PSUM") as pc_ps:
        wqk_sb = [pc_w.tile([128, 384], BF16, name=f"wqk{k}")
                  for k in range(DK)]
        wv_sb = [pc_w.tile([128, HC * HD], BF16, name=f"wv{k}")
                 for k in range(DK)]
        for k in range(DK):
            nc.sync.dma_start(wqk_sb[k][:],
                              dr["wqk"].ap()[128 * k:128 * (k + 1), :])
            nc.sync.dma_start(wv_sb[k][:],
                              dr["wv"].ap()[128 * k:128 * (k + 1), :])
        cct_sb = pc_w.tile([128, L], BF16, name="cct_sb")
        sst_sb = pc_w.tile([128, L], BF16, name="sst_sb")
        nc.sync.dma_start(cct_sb[:], dr["cct"].ap())
        nc.sync.dma_start(sst_sb[:], dr["sst"].ap())
        pswap_sb = pc_w.tile([128, 128], BF16, name="pswap_sb")
        nc.sync.dma_start(pswap_sb[:], dr["pswap"].ap())

        for m in range(3):
            qk_sb_m = pc.tile([128, L], BF16, name="qk_sb", bufs=2)
            for j in range(4):
                qkp = pc_ps.tile([128, 512], FP32, name="qkp")
                for k in range(DK):
                    nc.tensor.matmul(qkp[:],
                                     wqk_sb[k][:, 128 * m:128 * (m + 1)],
                                     hT[k][:, 512 * j:512 * (j + 1)],
                                     start=(k == 0), stop=(k == DK - 1))
                nc.vector.tensor_copy(qk_sb_m[:, 512 * j:512 * (j + 1)],
                                      qkp[:])
            t1 = pc.tile([128, L], BF16, name="t1", bufs=1)
            t2 = pc.tile([128, L], BF16, name="t2", bufs=1)
            nc.vector.tensor_mul(t1[:], qk_sb_m[:], cct_sb[:])
            for j in range(4):
                sl = slice(512 * j, 512 * (j + 1))
                swp = pc_ps.tile([128, 512], FP32, name="swp")
                nc.tensor.matmul(swp[:], pswap_sb[:], qk_sb_m[:, sl])
                nc.vector.tensor_mul(t2[:, sl], swp[:], sst_sb[:, sl])
            qkr_A, qkr_B, qkr_C, qkr_D = qkr
            if m == 0:      # (q0, q1) -> A full
                nc.vector.tensor_add(qkr_A[:], t1[:], t2[:])
            elif m == 1:    # (q2, k0) -> C[0:64], B[0:64]
                nc.vector.tensor_add(qkr_C[:], t1[0:64, :], t2[0:64, :])
                nc.vector.tensor_add(qkr_B[0:64, :], t1[64:128, :],
                                     t2[64:128, :])
            else:           # (k1, k2) -> B[64:128], D[0:64]
                nc.vector.tensor_add(qkr_B[64:128, :], t1[0:64, :],
                                     t2[0:64, :])
                nc.vector.tensor_add(qkr_D[:], t1[64:128, :],
                                     t2[64:128, :])

        for t in range(KC):
            vp = pc_ps.tile([128, HC * HD], FP32, name="vp")
            for k in range(DK):
                nc.tensor.matmul(vp[:], hT[k][:, 128 * t:128 * (t + 1)],
                                 wv_sb[k][:],
                                 start=(k == 0), stop=(k == DK - 1))
            for h in range(HC):
                nc.vector.tensor_copy(v_sb[t][:, 65 * h:65 * h + 64],
                                      vp[:, 64 * h:64 * (h + 1)])
            nc.scalar.activation(v_sb[t][:, 64:195:65], st["ones3_f"][:],
                                 AF.Copy)


def _attention(nc, tc, st, qkr, v_sb, wo_sb, rs_in):
    """scoresT -> exp -> PV (ones-augmented) -> normalize -> Wo -> rs_in.
    Emits ReduceScatter for the first token half after j==1."""
    bcast = st["bcast"]
    with tc.tile_pool(name="at", bufs=2) as at, \
         tc.tile_pool(name="at_exp", bufs=6) as at_exp, \
         tc.tile_pool(name="at_sps", bufs=3, space="PSUM") as at_sps, \
         tc.tile_pool(name="at_ops", bufs=2, space="PSUM") as at_ops, \
         tc.tile_pool(name="at_bps", bufs=1, space="PSUM") as at_bps, \
         tc.tile_pool(name="at_wps", bufs=2, space="PSUM") as at_wps:
        qkr_A, qkr_B, qkr_C, qkr_D = qkr
        qsl = [lambda s: qkr_A[0:64, s], lambda s: qkr_A[64:128, s],
               lambda s: qkr_C[0:64, s]]
        ksl = [lambda s: qkr_B[0:64, s], lambda s: qkr_B[64:128, s],
               lambda s: qkr_D[0:64, s]]
        for j in range(JT):
            o_sb = [at.tile([64, 512], BF16, name=f"o{h}")
                    for h in range(HC)]
            for h in range(HC):
                opsum = at_ops.tile([128, 512], FP32, name="opsum")
                for kc in range(KC):
                    sps = at_sps.tile([128, 512], FP32, name="sps")
                    nc.tensor.matmul(
                        sps[:],
                        ksl[h](slice(128 * kc, 128 * (kc + 1))),
                        qsl[h](slice(512 * j, 512 * (j + 1))))
                    ex = at_exp.tile([128, 512], BF16, name="ex")
                    nc.scalar.activation(ex[:], sps[:], AF.Exp,
                                         scale=1.0 / SCALE)
                    nc.tensor.matmul(
                        opsum[0:65, :], v_sb[kc][:, 65 * h:65 * (h + 1)],
                        ex[:], start=(kc == 0), stop=(kc == KC - 1))
                recip = at.tile([1, 512], FP32R, name="recip")
                nc.vector.reciprocal(recip[:], opsum[64:65, :])
                bps = at_bps.tile([128, 512], FP32, name="bps")
                nc.tensor.matmul(bps[0:64, :], st["ones_r"][0:1, 0:64],
                                 recip[:])
                rb64 = at.tile([64, 512], FP32, name="rb64")
                nc.vector.tensor_copy(rb64[:], bps[0:64, :])
                nc.vector.tensor_mul(o_sb[h][:], opsum[0:64, :], rb64[:])
            for tc4 in range(4):
                tok = 512 * j + 128 * tc4
                out1 = at.tile([128, D], BF16, name="out1", bufs=3)
                for half in range(2):
                    wps = at_wps.tile([128, 384], FP32, name="wps")
                    for h in range(HC):
                        nc.tensor.matmul(
                            wps[:],
                            o_sb[h][:, 128 * tc4:128 * (tc4 + 1)],
                            wo_sb[h][:, 384 * half:384 * (half + 1)],
                            start=(h == 0), stop=(h == HC - 1))
                    nc.vector.tensor_mul(
                        out1[:, 384 * half:384 * (half + 1)], wps[:],
                        bcast[2][:, 384 * half:384 * (half + 1)])
                if tok < 1024:
                    nc.sync.dma_start(rs_in[0][tok:tok + 128, :], out1[:])
                else:
                    nc.sync.dma_start(rs_in[1][tok - 1024:tok - 896, :],
                                      out1[:])
            if j == 1:
                nc.gpsimd.collective_compute(
                    "ReduceScatter", ALU.add, replica_groups=GROUPS,
                    ins=[st["rs_in0"].opt()], outs=[st["rs_out0"].opt()])


def _ffn_half(nc, tc, st, x1, h2t, half):
    """rms stats + modulation + transpose for one 256-token half, then
    gate/hidden matmuls for that half."""
    bcast, ident = st["bcast"], st["ident_bf"]
    ts = (0, 1) if half == 0 else (2, 3)
    with tc.tile_pool(name=f"pfh{half}", bufs=2) as pf, \
         tc.tile_pool(name=f"pfh{half}_tps", bufs=2, space="PSUM") as pf_tps:
        for t in ts:
            sq = pf.tile([128, D], FP32, name="sq")
            nc.vector.tensor_mul(sq[:], x1[t][:], x1[t][:])
            ms = pf.tile([128, 1], FP32, name="ms")
            nc.vector.reduce_sum(ms[:], sq[:], axis=AX.X)
            sr = pf.tile([128, 1], FP32, name="sr")
            nc.scalar.activation(sr[:], ms[:], AF.Sqrt,
                                 bias=st["eps_sb"][:, 0:1], scale=1.0 / D)
            rv = pf.tile([128, 1], FP32, name="rv")
            nc.vector.reciprocal(rv[:], sr[:])
            h2a = pf.tile([128, D], FP32, name="h2a")
            nc.vector.tensor_scalar(h2a[:], x1[t][:], rv[:, 0:1], None,
                                    op0=ALU.mult)
            h2b = pf.tile([128, D], FP32, name="h2b")
            nc.vector.tensor_mul(h2b[:], h2a[:], bcast[3][:])
            h2c = pf.tile([128, D], BF16, name="h2c")
            nc.vector.tensor_add(h2c[:], h2b[:], bcast[4][:])
            for k in range(DK):
                tp = pf_tps.tile([128, 128], BF16, name="tp")
                nc.tensor.transpose(tp[:], h2c[:, 128 * k:128 * (k + 1)],
                                    ident[:])
                nc.vector.tensor_copy(h2t[k][:, 128 * t:128 * (t + 1)],
                                      tp[:])

    sl = slice(HTOK * half, HTOK * (half + 1))
    with tc.tile_pool(name=f"pfg{half}", bufs=2) as pfg, \
         tc.tile_pool(name=f"pfg{half}_ps", bufs=2, space="PSUM") as pf_gps:
        for m in range(MG):
            gp = pf_gps.tile([128, HTOK], FP32, name="gp")
            hp = pf_gps.tile([128, HTOK], FP32, name="hp")
            for k in range(DK):
                nc.tensor.matmul(gp[:], st["wg"][m][:, 128 * k:128 * (k + 1)],
                                 h2t[k][:, sl],
                                 start=(k == 0), stop=(k == DK - 1))
            for k in range(DK):
                nc.tensor.matmul(hp[:], st["wh"][m][:, 128 * k:128 * (k + 1)],
                                 h2t[k][:, sl],
                                 start=(k == 0), stop=(k == DK - 1))
            sg = pfg.tile([128, HTOK], BF16, name="sg")
            nc.scalar.activation(sg[:], gp[:], AF.Silu)
            nc.vector.tensor_mul(st["ghT"][m][:, sl], sg[:], hp[:])


def _ffn_out(nc, tc, st, x1, out_d):
    """ghT @ ffn_out + gated residual, per 128-token chunk."""
    bcast = st["bcast"]
    with tc.tile_pool(name="pfo", bufs=2) as pf, \
         tc.tile_pool(name="pfo_ps", bufs=1, space="PSUM") as pf_ops:
        fps = [[pf_ops.tile([128, 384], FP32, name=f"fps{t}_{hf}")
                for hf in range(2)] for t in range(4)]
        for k in range(MG):
            for t in range(4):
                for hf in range(2):
                    nc.tensor.matmul(
                        fps[t][hf][:],
                        st["ghT"][k][:, 128 * t:128 * (t + 1)],
                        st["wo_f"][k][:, 384 * hf:384 * (hf + 1)],
                        start=(k == 0), stop=(k == MG - 1))
        for t in range(4):
            ot = pf.tile([128, D], FP32, name="ot")
            for hf in range(2):
                tt = pf.tile([128, 384], FP32, name="tt")
                nc.vector.tensor_mul(tt[:], fps[t][hf][:],
                                     bcast[5][:, 384 * hf:384 * (hf + 1)])
                nc.vector.tensor_add(ot[:, 384 * hf:384 * (hf + 1)],
                                     tt[:],
                                     x1[t][:, 384 * hf:384 * (hf + 1)])
            nc.sync.dma_start(out_d.ap()[128 * t:128 * (t + 1), :], ot[:])


def _emit(nc, tc, dr, out_d):
    with tc.tile_pool(name="pers", bufs=1) as pers, \
         tc.tile_pool(name="dram", bufs=1, space="DRAM") as dram:
        st = {}
        st["ident_bf"] = pers.tile([128, 128], BF16, name="ident_bf")
        make_identity(nc, st["ident_bf"][:])
        ones_f = pers.tile([1, 128], FP32, name="ones_f")
        nc.vector.memset(ones_f[:], 1.0)
        st["ones_r"] = pers.tile([1, 128], FP32R, name="ones_r")
        nc.scalar.activation(st["ones_r"][:], ones_f[:], AF.Copy)
        st["ones_bf"] = pers.tile([1, 128], BF16, name="ones_bf")
        nc.scalar.activation(st["ones_bf"][:], ones_f[:], AF.Copy)
        onescol_f = pers.tile([128, 1], FP32, name="onescol_f")
        nc.vector.memset(onescol_f[:], 1.0)
        st["onescol_bf"] = pers.tile([128, 1], BF16, name="onescol_bf")
        nc.scalar.activation(st["onescol_bf"][:], onescol_f[:], AF.Copy)
        st["ones3_f"] = pers.tile([128, 3], FP32, name="ones3_f")
        nc.vector.memset(st["ones3_f"][:], 1.0)
        st["eps_sb"] = pers.tile([128, 1], FP32, name="eps_sb")
        nc.vector.memset(st["eps_sb"][:], EPS)

        st["mod_cols"] = pers.tile([128, 12], FP32, name="mod_cols")
        st["bcast"] = {m: pers.tile([128, D], FP32, name=f"bcast{m}")
                       for m in (2, 3, 4, 5)}
        st["ar_in"] = dram.tile([2, 6 * D], FP32, name="ar_in")
        st["ar_out"] = dram.tile([2, 6 * D], FP32, name="ar_out")
        st["rs_in0"] = dram.tile([L // 2, D], BF16, name="rs_in0")
        st["rs_in1"] = dram.tile([L // 2, D], BF16, name="rs_in1")
        st["rs_out0"] = dram.tile([HTOK, D], BF16, name="rs_out0")
        st["rs_out1"] = dram.tile([HTOK, D], BF16, name="rs_out1")

        _phase_a1(nc, tc, dr, st)

        with tc.tile_pool(name="p_x1", bufs=1) as p_x1:
            # prefetch residual slice into x1 (rs partial added in later)
            x1 = [p_x1.tile([128, D], FP32, name=f"x1_{t}") for t in range(4)]
            for t in range(4):
                nc.sync.dma_start(
                    x1[t][:], dr["x_slice"].ap()[128 * t:128 * (t + 1), :])

            with tc.tile_pool(name="p_qv", bufs=1) as p_qv:
                qkr_A = p_qv.tile([128, L], BF16, name="qkr_A")
                qkr_B = p_qv.tile([128, L], BF16, name="qkr_B")
                qkr_C = p_qv.tile([64, L], BF16, name="qkr_C")
                qkr_D = p_qv.tile([64, L], BF16, name="qkr_D")
                qkr = (qkr_A, qkr_B, qkr_C, qkr_D)
                v_sb = [p_qv.tile([128, 200], BF16, name=f"v_sb{t}")
                        for t in range(KC)]
                wo_sb = [p_qv.tile([64, D], BF16, name=f"wo{h}")
                         for h in range(HC)]
                for h in range(HC):
                    nc.sync.dma_start(wo_sb[h][:],
                                      dr["wo"].ap()[64 * h:64 * (h + 1), :])
                with tc.tile_pool(name="p_h", bufs=1) as p_h:
                    hT = [p_h.tile([128, L], BF16, name=f"hT{k}")
                          for k in range(DK)]
                    with tc.tile_pool(name="p_xt", bufs=1) as p_xt:
                        xt = [p_xt.tile([128, L], BF16, name=f"xt{k}")
                              for k in range(DK)]
                        rb = p_xt.tile([128, L], BF16, name="rb")
                        _phase_b_stats(nc, tc, dr, st, xt, rb)
                        _phase_a2(nc, tc, dr, pers, st)
                        _phase_b_ht(nc, tc, st, xt, rb, hT)
                    _phase_c(nc, tc, dr, st, hT, qkr, v_sb)
                _attention(nc, tc, st, qkr, v_sb, wo_sb,
                           (st["rs_in0"], st["rs_in1"]))

            # FFN weights -> SBUF (streams during attention tail + RS)
            with tc.tile_pool(name="pfw", bufs=1) as pfw:
                st["wg"] = [pfw.tile([128, D], BF16, name=f"wg{m}")
                            for m in range(MG)]
                st["wh"] = [pfw.tile([128, D], BF16, name=f"wh{m}")
                            for m in range(MG)]
                st["wo_f"] = [pfw.tile([128, D], BF16, name=f"wof{m}")
                              for m in range(MG)]
                st["ghT"] = [pfw.tile([128, TOK], BF16, name=f"ghT{m}")
                             for m in range(MG)]
                for m in range(MG):
                    nc.sync.dma_start(
                        st["wg"][m][:],
                        dr["wg_blk"].ap()[128 * m:128 * (m + 1), :])
                    nc.sync.dma_start(
                        st["wh"][m][:],
                        dr["wh_blk"].ap()[128 * m:128 * (m + 1), :])
                for m in range(MG):
                    nc.sync.dma_start(
                        st["wo_f"][m][:],
                        dr["wout"].ap()[128 * m:128 * (m + 1), :])

                nc.gpsimd.collective_compute(
                    "ReduceScatter", ALU.add, replica_groups=GROUPS,
                    ins=[st["rs_in1"].opt()], outs=[st["rs_out1"].opt()])

                with tc.tile_pool(name="p_h2t", bufs=1) as p_h2t:
                    h2t = [p_h2t.tile([128, TOK], BF16, name=f"h2t{k}")
                           for k in range(DK)]
                    with tc.tile_pool(name="px0", bufs=2) as px0:
                        for t in (0, 1):
                            rsx = px0.tile([128, D], BF16, name="rsx")
                            nc.sync.dma_start(
                                rsx[:], st["rs_out0"][128 * t:128 * (t + 1), :])
                            nc.vector.tensor_add(x1[t][:], x1[t][:], rsx[:])
                    _ffn_half(nc, tc, st, x1, h2t, 0)
                    with tc.tile_pool(name="px1", bufs=2) as px1:
                        for t in (2, 3):
                            rsx = px1.tile([128, D], BF16, name="rsx")
                            nc.sync.dma_start(
                                rsx[:],
                                st["rs_out1"][128 * (t - 2):128 * (t - 1), :])
                            nc.vector.tensor_add(x1[t][:], x1[t][:], rsx[:])
                    _ffn_half(nc, tc, st, x1, h2t, 1)
                _ffn_out(nc, tc, st, x1, out_d)


# ---------------------------------------------------------------- entry
def get_program(reps=1):
    key = f"nc{reps}"
    if key not in _CACHE:
        _CACHE[key] = build_program(reps)
    return _CACHE[key]


def make_in_maps(inputs):
    cores = host_prep(inputs)
    names = [s[0] for s in DRAM_SPECS]
    return [{n: cores[i][n] for n in names} for i in range(NC_)]


def kernel(**inputs):
    nc = get_program()
    in_maps = make_in_maps(inputs)
    res = bass_utils.run_bass_kernel_spmd(nc, in_maps, list(range(NC_)))
    out = np.zeros((B, L, D), np.float32)
    for i in range(NC_):
        g, r = i // G, i % G
        o = res.results[i]["out"]
        out[g, HTOK * r:HTOK * (r + 1)] = o[:HTOK]
        out[g, L // 2 + HTOK * r:L // 2 + HTOK * (r + 1)] = o[HTOK:]
    return out
